# revision 34
# baseline (speedup 1.0000x reference)
"""Trainium2 Bass kernel for nn_CNN2 (time-lagged cross-correlation CNN).

Math note (exact algebraic identity, not an approximation):
  The reference computes Y = W @ ones(30, T), so every time-column of Y is
  r = W.sum(axis=1).  The full lagged cross-correlation is then
  S[lag] = count(lag) * r r^T, its trace is count(lag) * ||r||^2, so the
  per-lag trace-normalized matrix S_mean[lag] = r r^T / ||r||^2 is the SAME
  for every lag.  Hence mean-over-lags = r r^T/||r||^2 and var-over-lags = 0
  for ANY W and ANY T.  The kernel therefore computes
      Gm = 0.5*(r r^T/||r||^2 + 1),  Gv = 0.5
  followed by the CNN tail (conv 2->8 k4 p1, leaky 0.2, maxpool 8;
  conv 8->16 k2 p1, leaky, maxpool 4; linear 16->2), all on-device.

Implementation notes (this version):
  - The mask/constant channel of conv1 collapses to 4 rank-1 0/1 patterns
    over the 24x24 output grid.  Engine ops must start at partition 0, so
    the pattern rows are built as a nested memset cascade over partition
    prefixes (rows: const / dx|dy union / dx / corner) and the host does
    the corresponding basis change on the per-channel coefficients
    (conv1_b folded into the const term).  They enter the conv as ONE
    extra K=4 matmul per pool band.  No mask image, no conv1 bias matmuls.
  - Normalization is restructured OFF the critical path: conv1 runs
    UN-normalized (rank-1 image = r r^T raw; pattern coefs scaled by
    2*||r||^2, which is available early), and the 0.5/||r||^2 factor is
    folded into the conv2 weights (needed ~2us later).  leaky/maxpool are
    positively homogeneous, so the scale commutes through them.
  - The 4 row-shifted rank-1 images (conv rhs, kh on partitions) are built
    WITHOUT the fan-out DMA of the previous version: partition_all_reduce
    leaves r on every partition; 4 accumulating K=1 matmuls with host
    one-hot lhsT rows write the 4 shifted copies of r into PSUM (Q4), and
    one broadcast-multiply per 8-row band writes the fp16 images (two
    bands on DVE from PSUM, the last on Pool via an Act PSUM->SBUF copy,
    since Pool has no PSUM port).
  - conv1 output is split into THREE PSUM groups, one per maxpool row band,
    so each 8x8x3 pooling reduce (DVE, the only engine that can max from
    PSUM) starts as soon as its band's 5 matmuls (4 kw-slides + patterns)
    stop, pipelined behind the next band's matmuls.
  - conv2/linear biases ride as K=1 matmuls against ones rows; the linear
    result is copied PSUM->SBUF once and DMA'd out.
  - All weight-side packing/permutation is host-side; everything the DVE
    touches on the critical path is fp16 where the layout allows.
  - A chain of warm-up matmuls on memset data holds the PE p-state ramp so
    the real conv matmuls run at full clock.

The computation is replicated on the 8 NeuronCores (it is far below one
core's capacity; a cross-core split would only add collective latency), and
core 0's output is returned.
"""

import numpy as np

N = 30

_CACHE = {}


def _build_nc():
    from contextlib import ExitStack

    import concourse.bass as bass
    import concourse.tile as tile
    from concourse import bacc, bass_isa, mybir

    f32 = mybir.dt.float32
    f16 = mybir.dt.float16
    ALU = mybir.AluOpType
    AX = mybir.AxisListType

    nc = bacc.Bacc("TRN2")

    # W^T f32 in cols 0:30; cols 30:45 = W^T as fp16 pairs; cols 45:49 =
    # fp16 selector block (half-index 4 ones).  One DMA carries everything
    # the Q4 matmuls need, in fp16 for 1-cycle/row PE speed.
    wt_d = nc.dram_tensor("wt", [N, N + 19], f32, kind="ExternalInput")
    wp_d = nc.dram_tensor("wpack", [17, 64], f32, kind="ExternalInput")
    out_d = nc.dram_tensor("out", [1, 2], f32, kind="ExternalOutput")

    with tile.TileContext(nc) as tc, ExitStack() as ctx:
        sb = ctx.enter_context(tc.tile_pool(name="sb", bufs=1))
        ps = ctx.enter_context(tc.tile_pool(name="ps", bufs=1, space="PSUM"))

        # ---- tiles (engine ops may only address partition-0-based slices)
        wtsel = sb.tile([N, N + 19], f32)
        wpack = sb.tile([17, 64], f32)
        ones16 = sb.tile([1, 16], f16)
        R4 = sb.tile([4, 768], f16)     # rank-1 images, kh on partitions
        PAT = sb.tile([4, 192], f16)    # patterns: const / dx|dy / dx / dxy
        Q4sb = sb.tile([4, 32], f32)    # SBUF copy of Q4 for the Pool band
        rrow = sb.tile([N, N], f32)     # r on every partition
        sq = sb.tile([1, N], f32)
        ss = sb.tile([1, 1], f32)       # ||r||^2
        ssb4 = sb.tile([4, 1], f32)
        inv = sb.tile([1, 1], f32)      # 1/||r||^2
        inv8 = sb.tile([8, 1], f32)
        patc = sb.tile([4, 8], f16)     # pattern coefs * 2*||r||^2
        w2s = sb.tile([8, 64], f16)     # conv2 lhsT, scaled by 0.5/||r||^2
        p1 = sb.tile([8, 9], f16)       # pooled conv1 (co, py*3+px)
        p1p = sb.tile([8, 25], f16)     # conv2 rhs: zero-padded 5x5
        hraw = sb.tile([16, 1], f32)
        hcol = sb.tile([16, 1], f16)
        res = sb.tile([1, 2], f32)

        ps_w = ps.tile([1, 384], f32)
        ps_q = ps.tile([4, 30], f32)    # Q4: shifted copies of rpad
        psg = [ps.tile([8, 192], f32, name=f"psg{g}") for g in range(3)]
        ps2 = ps.tile([16, 16], f32)
        ps3 = ps.tile([1, 2], f32)

        # ---- input DMAs (wt first: it gates everything; wpack consumers
        # all run later than its arrival behind wt on the HWDGE queue)
        nc.sync.dma_start(out=wtsel, in_=wt_d.ap())
        nc.scalar.dma_start(out=wpack, in_=wp_d.ap())
        wt = wtsel[0:30, 0:30]
        wt16 = wtsel[0:30, 30:45].bitcast(f16)     # [30, 30] W^T fp16
        # [30, 8] fp16 selector block: col 4 = ones, others 0; the slice
        # esel[:, 4-kh : 8-kh] is the [30, 4] one-hot-column stationary
        # that routes column sums of wt into Q4 partition kh
        esel = wtsel[0:30, 45:49].bitcast(f16)
        w1v = wpack[0:4, 0:16].bitcast(f16)        # [4(kh), (kw co)] fp16
        patraw = wpack[0:4, 16:20].bitcast(f16)    # [4(term), 8(co)]
        w2raw = wpack[0:8, 20:52].bitcast(f16)     # [8, 64] = 0.5*w2
        b2row = wpack[0:1, 52:60].bitcast(f16)     # [1, 16]
        owt = wpack[0:16, 60:61].bitcast(f16)      # [16, 2]
        obrow = wpack[0:1, 61:62].bitcast(f16)     # [1, 2]

        # ---- early memsets (no input dependency): small ones on Pool (it
        # must be free when wt lands for the reduce), the big R4 zero on DVE
        patr = PAT.rearrange("p (h w) -> p h w", h=8)   # [4, 8, 24]
        nc.gpsimd.memset(ones16, 1.0)
        # pattern cascade: partition-prefix rectangles build nested rows
        # (row0 const, row1 dx|dy, row2 dx, row3 dxy); layout h*24+x
        nc.gpsimd.memset(PAT, 0.0)
        nc.gpsimd.memset(PAT[0:4, 0:1], 1.0)         # col 0: all rows
        nc.gpsimd.memset(patr[0:3, :, 0:1], 1.0)     # dx column: rows 0-2
        nc.gpsimd.memset(PAT[0:2, 0:24], 1.0)        # dy row: rows 0-1
        nc.gpsimd.memset(PAT[0:1, :], 1.0)           # const: row 0
        nc.vector.memset(R4, 0.0)
        nc.vector.memset(p1p, 0.0)

        # ---- PE warm-up chain on ones16 (ready ~0.8us): starts the
        # p-state ramp so the real conv matmuls run at full clock.
        nc.tensor.matmul(ps_w[0:1, 0:16], ones16[0:1, 0:1],
                         ones16[0:1, 0:16], start=True, stop=True)
        for _ in range(6):
            wrhs = bass.AP(ones16.tensor, ones16.offset,
                           [ones16.ap[0], [0, 24], [1, 16]])
            nc.tensor.matmul(ps_w, ones16[0:1, 0:1], wrhs,
                             start=True, stop=True)

        # ---- r = row sums of W, replicated on all 30 partitions (Pool);
        # used for the band in1, ||r||^2, and nothing else
        nc.gpsimd.partition_all_reduce(rrow, wt, N, bass_isa.ReduceOp.add)

        # ---- Q4[p, h] = rpad[h+p] (rpad = [0, r]) straight from wt in
        # PSUM: 4 accumulating K=30 matmuls; the [30, 4] one-hot-column
        # stationary (a slice of esel) routes sum_j wt[j, :] into partition
        # kh, with the rhs column slice providing the kh shift.  kh=1 goes
        # first (start=True zeroes all 30 cols); the kh=0 row writes cols
        # 1: so col 0 keeps the zero = the rpad leading 0.
        nc.tensor.matmul(ps_q[0:4, 0:30], esel[0:30, 3:7],
                         wt16[0:30, 0:30], start=True, stop=False)
        nc.tensor.matmul(ps_q[0:4, 1:30], esel[0:30, 4:8],
                         wt16[0:30, 0:29], start=False, stop=False)
        nc.tensor.matmul(ps_q[0:4, 0:29], esel[0:30, 2:6],
                         wt16[0:30, 1:30], start=False, stop=False)
        nc.tensor.matmul(ps_q[0:4, 0:28], esel[0:30, 1:5],
                         wt16[0:30, 2:30], start=False, stop=True)
        # SBUF copy for the Pool band (Pool has no PSUM port)
        nc.scalar.copy(Q4sb[0:4, 0:28], ps_q[0:4, 0:28])

        # ---- ||r||^2 chain (accumulate on DVE; Pool has no TensorScalarPtr)
        nc.vector.scalar_tensor_tensor(sq, rrow[0:1, :], 1.0, rrow[0:1, :],
                                       ALU.mult, ALU.mult, accum_out=ss)
        nc.gpsimd.partition_broadcast(ssb4, ss)
        # pattern coefs * ||r||^2 (host pre-doubled; un-normalized scale)
        ssb4b = bass.AP(ssb4.tensor, ssb4.offset, [[ssb4.ap[0][0], 4], [0, 8]])
        nc.gpsimd.tensor_mul(patc, patraw, ssb4b)

        # ---- rank-1 images: R4[p, h*32+x] = rpad[h+p] * rpad[x], written
        # in three 8-row bands (fp16 out).  in0 = Q4 (h varies, x b-cast),
        # in1 = rrow rows 0-3 (x varies, h b-cast).  Emission order matters:
        # per-engine queues are in-order, so the bands go ahead of the
        # off-critical-path inverse chain.
        pstride_rr = rrow.ap[0][0]
        pstride_R = R4.ap[0][0]

        def band_mult(eng, h0, qt):
            out = bass.AP(R4.tensor, R4.offset + h0 * 32 + 1,
                          [[pstride_R, 4], [32, 8], [1, 30]])
            q = bass.AP(qt.tensor, qt.offset + h0,
                        [[qt.ap[0][0], 4], [1, 8], [0, 30]])
            v = bass.AP(rrow.tensor, rrow.offset,
                        [[pstride_rr, 4], [0, 8], [1, 30]])
            eng.tensor_mul(out, q, v)

        band_mult(nc.vector, 0, ps_q)
        band_mult(nc.vector, 8, ps_q)
        band_mult(nc.gpsimd, 16, Q4sb)

        # ---- conv1: three 8-row PSUM groups.  The pattern matmuls (K=4,
        # start=True) are emitted first so they run as soon as patc lands,
        # before the rank-1 images exist; group 0 reads the h-resolved
        # pattern block, groups 1-2 re-read the h>=1 row with h-stride 0.
        pstride_P = PAT.ap[0][0]
        nc.tensor.matmul(psg[0], patc[0:4, 0:8],
                         patr[0:4, 0:8, 0:24], start=True, stop=False)
        for g in (1, 2):
            prhs = bass.AP(PAT.tensor, PAT.offset + 24,
                           [[pstride_P, 4], [0, 8], [1, 24]])
            nc.tensor.matmul(psg[g], patc[0:4, 0:8], prhs,
                             start=True, stop=False)
        # 4 kw-slide matmuls (K=4) per group accumulate on top; the 8x8x3
        # max-pool reduce (DVE) runs as soon as its group stops, pipelined
        # behind the next group's matmuls.  Group order g0, g2, g1 matches
        # band readiness (DVE band h0, Pool band h16, DVE band h8).
        group_order = (0, 2, 1)
        for g in group_order:
            for kw in range(4):
                rhs = bass.AP(R4.tensor, R4.offset + g * 256 + kw,
                              [[pstride_R, 4], [32, 8], [1, 24]])
                nc.tensor.matmul(psg[g], w1v[0:4, kw * 8:(kw + 1) * 8], rhs,
                                 start=False, stop=(kw == 3))
        for g in group_order:
            vg = psg[g].rearrange("p (h pc w) -> p pc h w", h=8, pc=3)
            nc.vector.tensor_reduce(p1[:, g * 3:(g + 1) * 3], vg,
                                    axis=AX.XY, op=ALU.max)

        # ---- leaky into the zero-padded conv2 rhs
        p13 = p1.rearrange("p (py px) -> p py px", py=3)
        p1v = p1p.rearrange("p (h w) -> p h w", h=5)
        nc.vector.scalar_tensor_tensor(p1v[0:8, 1:4, 1:4], p13, 0.2, p13,
                                       ALU.mult, ALU.max)

        # ---- conv2: 8->16, k2, pad 1 -> (16, 4, 4); bias via K=1 ones mm
        w2v = w2s.rearrange("p (pos co) -> p pos co", pos=4)
        for i in range(4):
            kh, kw = divmod(i, 2)
            nc.tensor.matmul(ps2, w2v[0:8, i, :],
                             p1v[0:8, kh:kh + 4, kw:kw + 4],
                             start=(i == 0), stop=False)
        nc.tensor.matmul(ps2, b2row, ones16[0:1, 0:16],
                         start=False, stop=True)

        # ---- maxpool 4x4 (whole map) + leaky -> hcol; linear + bias mm
        nc.vector.tensor_reduce(hraw, ps2, axis=AX.X, op=ALU.max)
        nc.vector.scalar_tensor_tensor(hcol, hraw, 0.2, hraw,
                                       ALU.mult, ALU.max)
        nc.tensor.matmul(ps3, hcol, owt, start=True, stop=False)
        nc.tensor.matmul(ps3, ones16[0:1, 0:1], obrow,
                         start=False, stop=True)
        nc.vector.tensor_copy(res, ps3)

        nc.sync.dma_start(out=out_d.ap(), in_=res)

        # ---- inverse chain + conv2-weight folding.  Emitted LAST so the
        # list scheduler gives it the lowest priority: it is off the
        # critical path (w2s is only needed by conv2, ~2us after its deps
        # resolve) and must not displace the bands/pools in the DVE/Pool
        # queues.  w2s runs on the otherwise-idle Act engine.
        nc.vector.reciprocal(inv, ss)
        nc.gpsimd.partition_broadcast(inv8, inv)
        nc.scalar.mul(w2s, w2raw, inv8)

    nc.compile()
    return nc


def _get_nc():
    if "nc" not in _CACHE:
        _CACHE["nc"] = _build_nc()
    return _CACHE["nc"]


def make_in_map(W, conv1_w, conv1_b, conv2_w, conv2_b, out_w, out_b):
    W = np.asarray(W, np.float32)
    conv1_w = np.asarray(conv1_w, np.float32)
    conv1_b = np.asarray(conv1_b, np.float32)
    conv2_w = np.asarray(conv2_w, np.float32)
    conv2_b = np.asarray(conv2_b, np.float32)
    out_w = np.asarray(out_w, np.float32)
    out_b = np.asarray(out_b, np.float32)

    def f16pack(a):
        h = np.ascontiguousarray(a.astype(np.float16))
        return h.view(np.float32)

    wtsel = np.zeros((N, N + 19), np.float32)
    wtsel[:, 0:N] = W.T
    wth = np.zeros((N, 30), np.float16)
    wth[:] = W.T.astype(np.float16)
    wtsel[:, N:N + 15] = wth.view(np.float32)
    sel = np.zeros((N, 8), np.float16)
    sel[:, 4] = 1.0
    wtsel[:, N + 15:N + 19] = sel.view(np.float32)
    wpack = np.zeros((17, 64), np.float32)

    # conv1 rank-1 lhsT: [kh, (kw, co)] fp16 raw (un-normalized)
    w1m = conv1_w[:, 0]                                  # (co, kh, kw)
    w1v = np.ascontiguousarray(w1m.transpose(1, 2, 0)).reshape(4, 32)
    wpack[0:4, 0:16] = f16pack(w1v)

    # mask-channel + bias pattern coefficients (device scales by 2*||r||^2):
    # F[co, y, x] = conv(w1sum, 0.5*mask)[y,x] + b1[co]
    #            = A0 + Ay*d(y=0) + Ax*d(x=0) + Axy*d(y=0)d(x=0)
    # expressed in the nested pattern basis (const, dx|dy, dx, dxy):
    #   F = A0*P0 + Ay*P1 + (Ax-Ay)*P2 + (Axy+Ay)*P3
    w1s = conv1_w.sum(axis=1)                            # (co, kh, kw)
    A0 = 0.5 * w1s.sum(axis=(1, 2)) + conv1_b
    Ay = -0.5 * w1s[:, 0, :].sum(axis=1)
    Ax = -0.5 * w1s[:, :, 0].sum(axis=1)
    Axy = 0.5 * w1s[:, 0, 0]
    wpack[0:4, 16:20] = f16pack(
        2.0 * np.stack([A0, Ay, Ax - Ay, Axy + Ay]))

    # conv2 lhsT rows = 0.5*w2 (device multiplies by 1/||r||^2 to complete
    # the 0.5/||r||^2 normalization); bias separate
    w2p = 0.5 * conv2_w.transpose(1, 2, 3, 0).reshape(8, 64)
    wpack[0:8, 20:52] = f16pack(w2p)
    wpack[0:1, 52:60] = f16pack(conv2_b.reshape(1, 16))

    # linear: out_w^T fp16, bias row fp16
    wpack[0:16, 60:61] = f16pack(out_w.T.copy())
    wpack[0:1, 61:62] = f16pack(out_b.reshape(1, 2))

    return {"wt": wtsel, "wpack": wpack}


def kernel(x=None, W=None, conv1_w=None, conv1_b=None, conv2_w=None,
           conv2_b=None, out_w=None, out_b=None, col=None, **_unused):
    from concourse.bass_utils import run_bass_kernel_spmd

    nc = _get_nc()
    in_map = make_in_map(W, conv1_w, conv1_b, conv2_w, conv2_b, out_w, out_b)
    n_cores = 8
    res = run_bass_kernel_spmd(nc, [in_map] * n_cores,
                               core_ids=list(range(n_cores)))
    out = np.asarray(res.results[0]["out"], np.float32).reshape(1, 2)
    return out


# revision 43
# speedup vs baseline: 1.0402x; 1.0402x over previous
"""Trainium2 Bass kernel for nn_CNN2 (time-lagged cross-correlation CNN).

Math note (exact algebraic identity, not an approximation):
  The reference computes Y = W @ ones(30, T), so every time-column of Y is
  r = W.sum(axis=1).  The full lagged cross-correlation is then
  S[lag] = count(lag) * r r^T, its trace is count(lag) * ||r||^2, so the
  per-lag trace-normalized matrix S_mean[lag] = r r^T / ||r||^2 is the SAME
  for every lag.  Hence mean-over-lags = r r^T/||r||^2 and var-over-lags = 0
  for ANY W and ANY T.  The kernel therefore computes
      Gm = 0.5*(r r^T/||r||^2 + 1),  Gv = 0.5
  followed by the CNN tail (conv 2->8 k4 p1, leaky 0.2, maxpool 8;
  conv 8->16 k2 p1, leaky, maxpool 4; linear 16->2), all on-device.

Implementation notes (this version):
  - The mask/constant channel of conv1 collapses to 4 rank-1 0/1 patterns
    over the 24x24 output grid.  Engine ops must start at partition 0, so
    the pattern rows are built as a nested memset cascade over partition
    prefixes (rows: const / dx|dy union / dx / corner) and the host does
    the corresponding basis change on the per-channel coefficients
    (conv1_b folded into the const term).  They enter the conv as ONE
    extra K=4 matmul per pool band.  No mask image, no conv1 bias matmuls.
  - Normalization is restructured OFF the critical path: conv1 runs
    UN-normalized (rank-1 image = r r^T raw; pattern coefs scaled by
    2*||r||^2, which is available early), and the 0.5/||r||^2 factor is
    folded into the conv2 weights (needed ~2us later).  leaky/maxpool are
    positively homogeneous, so the scale commutes through them.
  - The 4 row-shifted rank-1 images (conv rhs, kh on partitions) are built
    WITHOUT the fan-out DMA of the previous version: partition_all_reduce
    leaves r on every partition; 4 accumulating K=1 matmuls with host
    one-hot lhsT rows write the 4 shifted copies of r into PSUM (Q4), and
    one broadcast-multiply per 8-row band writes the fp16 images (two
    bands on DVE from PSUM, the last on Pool via an Act PSUM->SBUF copy,
    since Pool has no PSUM port).
  - conv1 output is split into THREE PSUM groups, one per maxpool row band,
    so each 8x8x3 pooling reduce (DVE, the only engine that can max from
    PSUM) starts as soon as its band's 5 matmuls (4 kw-slides + patterns)
    stop, pipelined behind the next band's matmuls.
  - conv2/linear biases ride as K=1 matmuls against ones rows; the linear
    result is copied PSUM->SBUF once and DMA'd out.
  - All weight-side packing/permutation is host-side; everything the DVE
    touches on the critical path is fp16 where the layout allows.
  - A chain of warm-up matmuls on memset data holds the PE p-state ramp so
    the real conv matmuls run at full clock.

The computation is replicated on the 8 NeuronCores (it is far below one
core's capacity; a cross-core split would only add collective latency), and
core 0's output is returned.
"""

import numpy as np

N = 30

_CACHE = {}


def _build_nc():
    from contextlib import ExitStack

    import concourse.bass as bass
    import concourse.tile as tile
    from concourse import bacc, bass_isa, mybir

    f32 = mybir.dt.float32
    f16 = mybir.dt.float16
    ALU = mybir.AluOpType
    AX = mybir.AxisListType

    nc = bacc.Bacc("TRN2")

    # W^T f32 in cols 0:30; cols 30:45 = W^T as fp16 pairs; cols 45:49 =
    # fp16 selector block (half-index 4 ones).  One DMA carries everything
    # the Q4 matmuls need, in fp16 for 1-cycle/row PE speed.
    wt_d = nc.dram_tensor("wt", [N, N + 23], f32, kind="ExternalInput")
    wp_d = nc.dram_tensor("wpack", [17, 64], f32, kind="ExternalInput")
    out_d = nc.dram_tensor("out", [1, 2], f32, kind="ExternalOutput")

    with tile.TileContext(nc) as tc, ExitStack() as ctx:
        sb = ctx.enter_context(tc.tile_pool(name="sb", bufs=1))
        ps = ctx.enter_context(tc.tile_pool(name="ps", bufs=1, space="PSUM"))

        # ---- tiles (engine ops may only address partition-0-based slices)
        wtsel = sb.tile([N, N + 23], f32)
        wpack = sb.tile([17, 64], f32)
        ones16 = sb.tile([1, 16], f16)
        # rank-1 images, kh on partitions: ONE TILE PER POOL GROUP so the
        # (tile-granular) dependency tracker lets group g's matmuls start
        # as soon as ITS band is written, not all three
        R4g = [sb.tile([4, 256], f16, name=f"r4g{g}") for g in range(3)]
        PAT = sb.tile([4, 192], f16)    # patterns: const / dx|dy / dx / dxy
        Q4sb = sb.tile([4, 32], f32)    # SBUF copy of Q4 (bands, Pool band)
        rrow = sb.tile([N, N], f32)     # r on every partition
        sq = sb.tile([1, N], f32)
        ss = sb.tile([1, 1], f32)       # ||r||^2
        ssb4 = sb.tile([4, 1], f32)
        inv = sb.tile([1, 1], f32)      # 1/||r||^2
        inv8 = sb.tile([8, 1], f32)
        patc = sb.tile([4, 8], f16)     # pattern coefs * 2*||r||^2
        w2s = sb.tile([8, 64], f16)     # conv2 lhsT, scaled by 0.5/||r||^2
        p1 = sb.tile([8, 9], f16)       # pooled conv1 (co, py*3+px)
        p1p = sb.tile([8, 25], f16)     # conv2 rhs: zero-padded 5x5
        hraw = sb.tile([16, 1], f32)
        hcol = sb.tile([16, 1], f16)
        res = sb.tile([1, 2], f32)

        ps_w = ps.tile([1, 384], f32)
        ps_q = ps.tile([4, 30], f32)    # Q4: shifted copies of rpad
        psg = [ps.tile([8, 192], f32, name=f"psg{g}") for g in range(3)]
        ps2 = ps.tile([16, 16], f32)
        ps3 = ps.tile([1, 2], f32)

        # ---- input DMAs (wt first: it gates everything; wpack consumers
        # all run later than its arrival behind wt on the HWDGE queue)
        nc.sync.dma_start(out=wtsel, in_=wt_d.ap())
        nc.scalar.dma_start(out=wpack, in_=wp_d.ap())
        wt = wtsel[0:30, 0:30]
        wt16 = wtsel[0:30, 30:45].bitcast(f16)     # [30, 30] W^T fp16
        # [30, 8] fp16 selector block: col 4 = ones, others 0; the slice
        # esel[:, 4-kh : 8-kh] is the [30, 4] one-hot-column stationary
        # that routes column sums of wt into Q4 partition kh
        esel = wtsel[0:30, 45:49].bitcast(f16)
        patraw = wtsel[0:4, 49:53].bitcast(f16)    # [4(term), 8(co)]
        w1v = wpack[0:4, 0:16].bitcast(f16)        # [4(kh), (kw co)] fp16
        w2raw = wpack[0:8, 20:52].bitcast(f16)     # [8, 64] = 0.5*w2
        b2row = wpack[0:1, 52:60].bitcast(f16)     # [1, 16]
        owt = wpack[0:16, 60:61].bitcast(f16)      # [16, 2]
        obrow = wpack[0:1, 61:62].bitcast(f16)     # [1, 2]

        # ---- early memsets (no input dependency): small ones on Pool (it
        # must be free when wt lands for the reduce), the big R4 zero on DVE
        patr = PAT.rearrange("p (h w) -> p h w", h=8)   # [4, 8, 24]
        nc.gpsimd.memset(ones16, 1.0)
        # pattern cascade: partition-prefix rectangles build nested rows
        # (row0 const, row1 dx|dy, row2 dx, row3 dxy); layout h*24+x
        nc.gpsimd.memset(PAT, 0.0)
        nc.gpsimd.memset(PAT[0:4, 0:1], 1.0)         # col 0: all rows
        nc.gpsimd.memset(patr[0:3, :, 0:1], 1.0)     # dx column: rows 0-2
        nc.gpsimd.memset(PAT[0:2, 0:24], 1.0)        # dy row: rows 0-1
        nc.gpsimd.memset(PAT[0:1, :], 1.0)           # const: row 0
        nc.vector.memset(R4g[0], 0.0)
        nc.vector.memset(R4g[1], 0.0)
        nc.gpsimd.memset(R4g[2], 0.0)
        nc.vector.memset(p1p, 0.0)

        # ---- PE warm-up chain on ones16 (ready ~0.8us): starts the
        # p-state ramp so the real conv matmuls run at full clock.
        nc.tensor.matmul(ps_w[0:1, 0:16], ones16[0:1, 0:1],
                         ones16[0:1, 0:16], start=True, stop=True)
        for _ in range(6):
            wrhs = bass.AP(ones16.tensor, ones16.offset,
                           [ones16.ap[0], [0, 24], [1, 16]])
            nc.tensor.matmul(ps_w, ones16[0:1, 0:1], wrhs,
                             start=True, stop=True)

        # ---- r = row sums of W, replicated on all 30 partitions (Pool);
        # used for the band in1, ||r||^2, and nothing else
        nc.gpsimd.partition_all_reduce(rrow, wt, N, bass_isa.ReduceOp.add)

        # ---- Q4[p, h] = rpad[h+p] (rpad = [0, r]) straight from wt in
        # PSUM: 4 accumulating K=30 matmuls; the [30, 4] one-hot-column
        # stationary (a slice of esel) routes sum_j wt[j, :] into partition
        # kh, with the rhs column slice providing the kh shift.  kh=1 goes
        # first (start=True zeroes all 30 cols); the kh=0 row writes cols
        # 1: so col 0 keeps the zero = the rpad leading 0.
        nc.tensor.matmul(ps_q[0:4, 0:30], esel[0:30, 3:7],
                         wt16[0:30, 0:30], start=True, stop=False)
        nc.tensor.matmul(ps_q[0:4, 1:30], esel[0:30, 4:8],
                         wt16[0:30, 0:29], start=False, stop=False)
        nc.tensor.matmul(ps_q[0:4, 0:29], esel[0:30, 2:6],
                         wt16[0:30, 1:30], start=False, stop=False)
        nc.tensor.matmul(ps_q[0:4, 0:28], esel[0:30, 1:5],
                         wt16[0:30, 2:30], start=False, stop=True)
        # SBUF copy feeding all three bands (Act is idle and PSUM-close;
        # SBUF operands shave the DVE PSUM-access penalty and let the Pool
        # band run without a PSUM port)
        nc.scalar.copy(Q4sb[0:4, 0:28], ps_q[0:4, 0:28])

        # ---- ||r||^2 chain (accumulate on DVE; Pool has no TensorScalarPtr)
        nc.vector.scalar_tensor_tensor(sq, rrow[0:1, :], 1.0, rrow[0:1, :],
                                       ALU.mult, ALU.mult, accum_out=ss)
        nc.gpsimd.partition_broadcast(ssb4, ss)
        # pattern coefs * ||r||^2 (host pre-doubled; un-normalized scale)
        ssb4b = bass.AP(ssb4.tensor, ssb4.offset, [[ssb4.ap[0][0], 4], [0, 8]])
        nc.gpsimd.tensor_mul(patc, patraw, ssb4b)

        # ---- rank-1 images: R4g[g][p, h*32+x] = rpad[8g+h+p] * rpad[x],
        # one 8-row band per pool group (fp16 out).  in0 = Q4sb (h varies,
        # x b-cast), in1 = rrow rows 0-3 (x varies, h b-cast).
        pstride_rr = rrow.ap[0][0]

        def band_mult(eng, g):
            t = R4g[g]
            out = bass.AP(t.tensor, t.offset + 1,
                          [[t.ap[0][0], 4], [32, 8], [1, 30]])
            q = bass.AP(Q4sb.tensor, Q4sb.offset + 8 * g,
                        [[Q4sb.ap[0][0], 4], [1, 8], [0, 30]])
            v = bass.AP(rrow.tensor, rrow.offset,
                        [[pstride_rr, 4], [0, 8], [1, 30]])
            eng.tensor_mul(out, q, v)

        band_mult(nc.vector, 0)
        band_mult(nc.gpsimd, 2)
        band_mult(nc.vector, 1)

        # ---- conv1: three 8-row PSUM groups.  The pattern matmuls (K=4,
        # start=True) are emitted first so they run as soon as patc lands,
        # before the rank-1 images exist; group 0 reads the h-resolved
        # pattern block, groups 1-2 re-read the h>=1 row with h-stride 0.
        pstride_P = PAT.ap[0][0]
        nc.tensor.matmul(psg[0], patc[0:4, 0:8],
                         patr[0:4, 0:8, 0:24], start=True, stop=False)
        for g in (1, 2):
            prhs = bass.AP(PAT.tensor, PAT.offset + 24,
                           [[pstride_P, 4], [0, 8], [1, 24]])
            nc.tensor.matmul(psg[g], patc[0:4, 0:8], prhs,
                             start=True, stop=False)
        # 4 kw-slide matmuls (K=4) per group accumulate on top; the 8x8x3
        # max-pool reduce (DVE) runs as soon as its group stops, pipelined
        # behind the next group's matmuls.  Group order g0, g2, g1 matches
        # band readiness (DVE band h0, Pool band h16, DVE band h8).
        group_order = (0, 2, 1)
        for g in group_order:
            t = R4g[g]
            for kw in range(4):
                rhs = bass.AP(t.tensor, t.offset + kw,
                              [[t.ap[0][0], 4], [32, 8], [1, 24]])
                nc.tensor.matmul(psg[g], w1v[0:4, kw * 8:(kw + 1) * 8], rhs,
                                 start=False, stop=(kw == 3))
        for g in group_order:
            vg = psg[g].rearrange("p (h pc w) -> p pc h w", h=8, pc=3)
            nc.vector.tensor_reduce(p1[:, g * 3:(g + 1) * 3], vg,
                                    axis=AX.XY, op=ALU.max)

        # ---- leaky into the zero-padded conv2 rhs
        p13 = p1.rearrange("p (py px) -> p py px", py=3)
        p1v = p1p.rearrange("p (h w) -> p h w", h=5)
        nc.vector.scalar_tensor_tensor(p1v[0:8, 1:4, 1:4], p13, 0.2, p13,
                                       ALU.mult, ALU.max)

        # ---- conv2: 8->16, k2, pad 1 -> (16, 4, 4); bias via K=1 ones mm
        w2v = w2s.rearrange("p (pos co) -> p pos co", pos=4)
        for i in range(4):
            kh, kw = divmod(i, 2)
            nc.tensor.matmul(ps2, w2v[0:8, i, :],
                             p1v[0:8, kh:kh + 4, kw:kw + 4],
                             start=(i == 0), stop=False)
        nc.tensor.matmul(ps2, b2row, ones16[0:1, 0:16],
                         start=False, stop=True)

        # ---- maxpool 4x4 (whole map) + leaky -> hcol; linear + bias mm
        nc.vector.tensor_reduce(hraw, ps2, axis=AX.X, op=ALU.max)
        nc.vector.scalar_tensor_tensor(hcol, hraw, 0.2, hraw,
                                       ALU.mult, ALU.max)
        nc.tensor.matmul(ps3, hcol, owt, start=True, stop=False)
        nc.tensor.matmul(ps3, ones16[0:1, 0:1], obrow,
                         start=False, stop=True)
        nc.vector.tensor_copy(res, ps3)

        nc.sync.dma_start(out=out_d.ap(), in_=res)

        # ---- inverse chain + conv2-weight folding.  Emitted LAST so the
        # list scheduler gives it the lowest priority: it is off the
        # critical path (w2s is only needed by conv2, ~2us after its deps
        # resolve) and must not displace the bands/pools in the DVE/Pool
        # queues.  w2s runs on the otherwise-idle Act engine.
        nc.vector.reciprocal(inv, ss)
        nc.gpsimd.partition_broadcast(inv8, inv)
        nc.scalar.mul(w2s, w2raw, inv8)

    nc.compile()
    return nc


def _get_nc():
    if "nc" not in _CACHE:
        _CACHE["nc"] = _build_nc()
    return _CACHE["nc"]


def make_in_map(W, conv1_w, conv1_b, conv2_w, conv2_b, out_w, out_b):
    W = np.asarray(W, np.float32)
    conv1_w = np.asarray(conv1_w, np.float32)
    conv1_b = np.asarray(conv1_b, np.float32)
    conv2_w = np.asarray(conv2_w, np.float32)
    conv2_b = np.asarray(conv2_b, np.float32)
    out_w = np.asarray(out_w, np.float32)
    out_b = np.asarray(out_b, np.float32)

    def f16pack(a):
        h = np.ascontiguousarray(a.astype(np.float16))
        return h.view(np.float32)

    wtsel = np.zeros((N, N + 23), np.float32)
    wtsel[:, 0:N] = W.T
    wth = np.zeros((N, 30), np.float16)
    wth[:] = W.T.astype(np.float16)
    wtsel[:, N:N + 15] = wth.view(np.float32)
    sel = np.zeros((N, 8), np.float16)
    sel[:, 4] = 1.0
    wtsel[:, N + 15:N + 19] = sel.view(np.float32)
    wpack = np.zeros((17, 64), np.float32)

    # conv1 rank-1 lhsT: [kh, (kw, co)] fp16 raw (un-normalized)
    w1m = conv1_w[:, 0]                                  # (co, kh, kw)
    w1v = np.ascontiguousarray(w1m.transpose(1, 2, 0)).reshape(4, 32)
    wpack[0:4, 0:16] = f16pack(w1v)

    # mask-channel + bias pattern coefficients (device scales by 2*||r||^2):
    # F[co, y, x] = conv(w1sum, 0.5*mask)[y,x] + b1[co]
    #            = A0 + Ay*d(y=0) + Ax*d(x=0) + Axy*d(y=0)d(x=0)
    # expressed in the nested pattern basis (const, dx|dy, dx, dxy):
    #   F = A0*P0 + Ay*P1 + (Ax-Ay)*P2 + (Axy+Ay)*P3
    w1s = conv1_w.sum(axis=1)                            # (co, kh, kw)
    A0 = 0.5 * w1s.sum(axis=(1, 2)) + conv1_b
    Ay = -0.5 * w1s[:, 0, :].sum(axis=1)
    Ax = -0.5 * w1s[:, :, 0].sum(axis=1)
    Axy = 0.5 * w1s[:, 0, 0]
    wtsel[0:4, N + 19:N + 23] = f16pack(
        2.0 * np.stack([A0, Ay, Ax - Ay, Axy + Ay]))

    # conv2 lhsT rows = 0.5*w2 (device multiplies by 1/||r||^2 to complete
    # the 0.5/||r||^2 normalization); bias separate
    w2p = 0.5 * conv2_w.transpose(1, 2, 3, 0).reshape(8, 64)
    wpack[0:8, 20:52] = f16pack(w2p)
    wpack[0:1, 52:60] = f16pack(conv2_b.reshape(1, 16))

    # linear: out_w^T fp16, bias row fp16
    wpack[0:16, 60:61] = f16pack(out_w.T.copy())
    wpack[0:1, 61:62] = f16pack(out_b.reshape(1, 2))

    return {"wt": wtsel, "wpack": wpack}


def kernel(x=None, W=None, conv1_w=None, conv1_b=None, conv2_w=None,
           conv2_b=None, out_w=None, out_b=None, col=None, **_unused):
    from concourse.bass_utils import run_bass_kernel_spmd

    nc = _get_nc()
    in_map = make_in_map(W, conv1_w, conv1_b, conv2_w, conv2_b, out_w, out_b)
    n_cores = 8
    res = run_bass_kernel_spmd(nc, [in_map] * n_cores,
                               core_ids=list(range(n_cores)))
    out = np.asarray(res.results[0]["out"], np.float32).reshape(1, 2)
    return out
